# revision 5
# baseline (speedup 1.0000x reference)
"""CompGCN 2-layer forward on 8 Trainium2 NeuronCores (Bass/Tile).

Strategy (dst-sharded):
- Core c owns destination nodes [c*12500, (c+1)*12500), split into 49 windows of 256.
- Edges bucketed by (window, direction, src-region); scatter-add done as
  one-hot matmuls on TensorE: aggT[feat, node] += msg[edge, feat].T @ S[edge, node]
  with S[e, n] = (dstl[e] == n) * norm[e] built on DVE (bf16).
- Per-edge GEMM moved after aggregation (linearity): res = agg @ W.
- x / rel rows gathered with dma_gather (int16 idx, 4 src regions for x).
- BN+tanh fused into one ScalarE activation per tile; BN stats via tiny AllReduce.
- One AllGather of the layer-1 output shard; final unshard on host.
"""
import os
import numpy as np
import ml_dtypes

import concourse.bacc as bacc
import concourse.bass as bass
import concourse.mybir as mybir
import concourse.tile as tile
from concourse.bass_utils import run_bass_kernel_spmd
from concourse.masks import make_identity

N = 100000
D = 200
DP = 256
R2 = 400
E = 400000
NCORE = 8
NSH = 12500
WIN = 256
NW = 49            # ceil(12500/256)
NSH_PAD = NW * WIN  # 12544
NREG = 4
REGSZ = 25600
GT = 8             # tiles per gather call
P = 128
BN_EPS = 1e-5
BF = ml_dtypes.bfloat16

f32 = mybir.dt.float32
f32r = mybir.dt.float32r
bf16 = mybir.dt.bfloat16
i16 = mybir.dt.int16


def _wrap16(idx):
    """logical gather position i -> [i%16, i//16], replicated to 8 Q7 stripes."""
    a = idx.reshape(-1, 16).T.astype(np.int16)
    return np.ascontiguousarray(np.tile(a, (8, 1)))


def _slotmajor(a, dtype):
    """slot i -> [i%128, i//128]"""
    return np.ascontiguousarray(a.reshape(-1, P).T.astype(dtype))


def _prep(edge_index, edge_type):
    """Build uniform-across-cores tile structure + per-core slot arrays."""
    dirs = []
    for d in range(2):
        sl = slice(0, E) if d == 0 else slice(E, 2 * E)
        src = edge_index[0, sl].astype(np.int64)
        dst = edge_index[1, sl].astype(np.int64)
        et = edge_type[sl].astype(np.int64)
        deg = np.bincount(src, minlength=N).astype(np.float64)
        dinv = np.where(deg > 0, deg ** -0.5, 0.0)
        norm = (dinv[src] * dinv[dst]).astype(np.float32)
        dirs.append((src, dst, et, norm))

    # counts[c, d, w, r]
    counts = np.zeros((NCORE, 2, NW, NREG), np.int64)
    per_dir_sorted = []
    for d, (src, dst, et, norm) in enumerate(dirs):
        c = dst // NSH
        w = (dst % NSH) // WIN
        r = src // REGSZ
        key = ((c * 2 + 0) * NW + w) * NREG + r  # d folded later; sort within dir
        seg = (c * NW + w) * NREG + r
        order = np.lexsort((dst, seg))
        counts[:, d, :, :] = np.bincount(seg, minlength=NCORE * NW * NREG).reshape(
            NCORE, NW, NREG)
        per_dir_sorted.append((order, seg[order]))

    T = np.ceil(counts.max(axis=0) / P).astype(np.int64)  # [2, NW, NREG]
    for w in range(NW):
        for d in range(2):
            if T[d, w].sum() == 0:
                T[d, w, 0] = 1

    # G order: (w, d, r); tile index bookkeeping
    TOTG = int(T.sum())
    TOTG_PAD = ((TOTG + GT - 1) // GT) * GT
    LENG = TOTG * P
    NT_r = [int(T[:, :, r].sum()) for r in range(NREG)]
    NT_r_pad = [((n + GT - 1) // GT) * GT for n in NT_r]

    # per-G-tile: region, x-stream index within region
    tile_region = np.zeros(TOTG, np.int64)
    tile_xidx = np.zeros(TOTG, np.int64)
    xctr = [0] * NREG
    t = 0
    for w in range(NW):
        for d in range(2):
            for r in range(NREG):
                for _ in range(int(T[d, w, r])):
                    tile_region[t] = r
                    tile_xidx[t] = xctr[r]
                    xctr[r] += 1
                    t += 1
    assert t == TOTG

    cores = []
    for c in range(NCORE):
        et_slot = np.zeros(LENG, np.int64)
        dstl_slot = np.zeros(LENG, np.float32)
        norm_slot = np.zeros(LENG, np.float32)
        src_slot = np.zeros(LENG, np.int64)
        pos = 0
        for w in range(NW):
            for d in range(2):
                src, dst, et, norm = dirs[d]
                order, segs = per_dir_sorted[d]
                for r in range(NREG):
                    nt = int(T[d, w, r])
                    if nt == 0:
                        continue
                    seg_id = (c * NW + w) * NREG + r
                    lo = np.searchsorted(segs, seg_id, "left")
                    hi = np.searchsorted(segs, seg_id, "right")
                    idxs = order[lo:hi]
                    n = hi - lo
                    assert n <= nt * P
                    src_slot[pos:pos + n] = src[idxs]
                    et_slot[pos:pos + n] = et[idxs]
                    dstl_slot[pos:pos + n] = (dst[idxs] - c * NSH - w * WIN)
                    norm_slot[pos:pos + n] = norm[idxs]
                    pos += nt * P
        assert pos == LENG

        # x idx per region stream (G-order subsequence), padded to GT tiles
        xidx = []
        for r in range(NREG):
            sel = np.concatenate([
                src_slot[tg * P:(tg + 1) * P] for tg in range(TOTG)
                if tile_region[tg] == r]) if NT_r[r] else np.zeros(0, np.int64)
            arr = np.zeros(NT_r_pad[r] * P, np.int64)
            arr[:len(sel)] = sel - r * REGSZ
            # pad slots in real tiles have src_slot=0 -> idx -r*REGSZ < 0! fix:
            # pad slots must gather row 0 of the region (idx 0, killed by S=0).
            flat_norm = np.concatenate([
                norm_slot[tg * P:(tg + 1) * P] for tg in range(TOTG)
                if tile_region[tg] == r]) if NT_r[r] else np.zeros(0, np.float32)
            arr[:len(sel)][flat_norm == 0] = 0
            arr = np.clip(arr, 0, REGSZ - 1)
            xidx.append(_wrap16(arr))

        et_pad = np.zeros(TOTG_PAD * P, np.int64)
        et_pad[:LENG] = et_slot
        cores.append(dict(
            xidx=xidx,
            etidx=_wrap16(et_pad),
            dstl=_slotmajor(dstl_slot, np.float32),
            norm=_slotmajor(norm_slot, np.float32),
        ))

    meta = dict(T=T, TOTG=TOTG, TOTG_PAD=TOTG_PAD, NT_r=NT_r, NT_r_pad=NT_r_pad,
                tile_region=tile_region, tile_xidx=tile_xidx)
    return meta, cores


def _build(meta):
    T = meta["T"]
    TOTG = meta["TOTG"]
    TOTG_PAD = meta["TOTG_PAD"]
    NT_r_pad = meta["NT_r_pad"]
    tile_region = meta["tile_region"]
    tile_xidx = meta["tile_xidx"]

    nc = bacc.Bacc("TRN2", target_bir_lowering=False, debug=False,
                   num_devices=NCORE)

    # ---- inputs ----
    x0bf_d = nc.dram_tensor("x0bf", [N, DP], bf16, kind="ExternalInput")
    x0own_d = nc.dram_tensor("x0own", [NSH_PAD, DP], bf16, kind="ExternalInput")
    rel1bf_d = nc.dram_tensor("rel1bf", [R2, DP], bf16, kind="ExternalInput")
    rel1f_d = nc.dram_tensor("rel1f", [R2, D], f32, kind="ExternalInput")
    w_d = {}
    for name in ["win1", "wout1", "wloop1", "wrel1", "win2", "wout2", "wloop2"]:
        w_d[name] = nc.dram_tensor(name, [D, D], f32r, kind="ExternalInput")
    lr1_d = nc.dram_tensor("lrel1", [D, 1], f32, kind="ExternalInput")
    lr2_d = nc.dram_tensor("lrel2", [D, 1], f32, kind="ExternalInput")
    xidx_d = [nc.dram_tensor(f"xidx{r}", [P, NT_r_pad[r] * 8], i16,
                             kind="ExternalInput") for r in range(NREG)]
    etidx_d = nc.dram_tensor("etidx", [P, TOTG_PAD * 8], i16, kind="ExternalInput")
    dstl_d = nc.dram_tensor("dstl", [P, TOTG], f32, kind="ExternalInput")
    norm_d = nc.dram_tensor("norm", [P, TOTG], f32, kind="ExternalInput")

    x2_d = nc.dram_tensor("x2", [NSH, D], f32, kind="ExternalOutput")

    # ---- internal DRAM ----
    ag_in = nc.dram_tensor("ag_in", [NSH_PAD, DP], bf16)
    x1full = nc.dram_tensor("x1full", [N, DP], bf16, addr_space="Shared")
    rel2bf = nc.dram_tensor("rel2bf", [R2, DP], bf16)
    stashA = nc.dram_tensor("stashA", [NW, P, DP], f32)
    stashB = nc.dram_tensor("stashB", [NW, 72, DP], f32)
    bn_in = [nc.dram_tensor(f"bn_in{l}", [DP, 2], f32) for l in range(2)]
    bn_out = [nc.dram_tensor(f"bn_out{l}", [DP, 2], f32, addr_space="Shared")
              for l in range(2)]
    rg = [list(range(NCORE))]

    KCH = [(0, 128), (128, 72)]  # feat chunks (offset, size)

    with tile.TileContext(nc) as tc:
        with (
            tc.tile_pool(name="const", bufs=1) as cp,
            tc.tile_pool(name="xg", bufs=3) as xgp,
            tc.tile_pool(name="rg", bufs=3) as rgp,
            tc.tile_pool(name="sS", bufs=4) as sp,
            tc.tile_pool(name="msg", bufs=4) as mp,
            tc.tile_pool(name="stg", bufs=3) as stg,
            tc.tile_pool(name="rows", bufs=3) as rwp,
            tc.tile_pool(name="pagg", bufs=2, space="PSUM") as pagg,
            tc.tile_pool(name="pres", bufs=1, space="PSUM") as pres,
            tc.tile_pool(name="ptp", bufs=2, space="PSUM") as ptp,
        ):
            # ---------------- prelude ----------------
            zt_bf = cp.tile([P, DP], bf16)
            nc.vector.memset(zt_bf[:], 0.0)
            zt_f = cp.tile([P, DP], f32)
            nc.vector.memset(zt_f[:], 0.0)
            # zero ag_in (incl. pad rows + pad cols)
            for t0 in range(0, NSH_PAD, P):
                nv = min(P, NSH_PAD - t0)
                nc.sync.dma_start(out=ag_in[t0:t0 + nv, :], in_=zt_bf[:nv, :])
            for t0 in range(0, R2, P):
                nv = min(P, R2 - t0)
                nc.sync.dma_start(out=rel2bf[t0:t0 + nv, :], in_=zt_bf[:nv, :])
            for l in range(2):
                nc.sync.dma_start(out=bn_in[l][0:128, :], in_=zt_f[:, :2])
                nc.sync.dma_start(out=bn_in[l][128:256, :], in_=zt_f[:, :2])

            eps_t = cp.tile([P, 1], f32)
            nc.vector.memset(eps_t[:], BN_EPS)
            iota = cp.tile([P, DP], f32)
            nc.gpsimd.iota(iota[:], pattern=[[1, DP]], base=0,
                           channel_multiplier=0,
                           allow_small_or_imprecise_dtypes=True)
            ident_bf = cp.tile([P, P], bf16)
            make_identity(nc, ident_bf[:])
            ident_f = cp.tile([P, P], f32)
            make_identity(nc, ident_f[:])

            W = {}
            for name, dd in w_d.items():
                a = cp.tile([P, D], f32r, tag=f"W{name}a")
                b = cp.tile([72, D], f32r, tag=f"W{name}b")
                nc.sync.dma_start(out=a[:], in_=dd[0:128, :])
                nc.sync.dma_start(out=b[:], in_=dd[128:200, :])
                W[name] = (a, b)
            LR = []
            for dd in [lr1_d, lr2_d]:
                a = cp.tile([P, 1], f32, tag="lra")
                b = cp.tile([72, 1], f32, tag="lrb")
                nc.sync.dma_start(out=a[:], in_=dd[0:128, :])
                nc.sync.dma_start(out=b[:], in_=dd[128:200, :])
                LR.append((a, b))

            xidx_t = []
            for r in range(NREG):
                it = cp.tile([P, NT_r_pad[r] * 8], i16, tag=f"xidx{r}")
                nc.sync.dma_start(out=it[:], in_=xidx_d[r][:, :])
                xidx_t.append(it)
            etidx_t = cp.tile([P, TOTG_PAD * 8], i16)
            nc.sync.dma_start(out=etidx_t[:], in_=etidx_d[:, :])
            dstl_t = cp.tile([P, TOTG], f32)
            nc.sync.dma_start(out=dstl_t[:], in_=dstl_d[:, :])
            norm_t = cp.tile([P, TOTG], f32)
            nc.sync.dma_start(out=norm_t[:], in_=norm_d[:, :])

            # BN stats tiles
            stats = []
            for l in range(2):
                stats.append({
                    nm: cp.tile([csz, NW], f32, tag=f"{nm}{l}",
                                name=f"stat_{nm}{l}")
                    for nm, csz in [("sA", P), ("sB", 72),
                                    ("qA", P), ("qB", 72)]
                })

            # ---------------- per layer ----------------
            for l in range(2):
                xtable = x0bf_d if l == 0 else x1full
                reltable = rel1bf_d if l == 0 else rel2bf
                xown = x0own_d if l == 0 else ag_in
                wi, wo, wl = (("win1", "wout1", "wloop1") if l == 0
                              else ("win2", "wout2", "wloop2"))

                if l == 1:
                    # rel2 = init_rel @ w_rel1  (row-major bf16 into rel2bf)
                    for rc in range(4):
                        r0 = rc * 100
                        rl = stg.tile([100, D], f32, tag="r2in")
                        nc.sync.dma_start(out=rl[:], in_=rel1f_d[r0:r0 + 100, :])
                        rT = []
                        for (c0, csz) in KCH:
                            tp = ptp.tile([P, P], f32, tag="ptp")
                            nc.tensor.transpose(out=tp[:csz, :100],
                                                in_=rl[:, c0:c0 + csz],
                                                identity=ident_f[:100, :100])
                            sbT = stg.tile([P, 100], f32r, tag=f"r2T{c0}")
                            nc.scalar.copy(out=sbT[:csz, :], in_=tp[:csz, :100])
                            rT.append((sbT, csz))
                        outs = []
                        for (m0, msz) in KCH:
                            op = pres.tile([P, 100], f32, tag="presA")
                            for ki, ((k0, ksz), (rhs, _)) in enumerate(
                                    zip(KCH, rT)):
                                nc.tensor.matmul(
                                    out=op[:msz, :],
                                    lhsT=W["wrel1"][ki][:, m0:m0 + msz],
                                    rhs=rhs[:ksz, :],
                                    start=(ki == 0), stop=(ki == 1))
                            sb2 = stg.tile([P, 100], f32, tag=f"r2o{m0}")
                            nc.scalar.copy(out=sb2[:msz, :], in_=op[:msz, :])
                            outs.append((sb2, msz))
                        rows = rwp.tile([100, DP], bf16, tag="r2rows")
                        for (sb2, msz), (m0, _) in zip(outs, KCH):
                            tpf = ptp.tile([P, P], f32, tag="ptp")
                            nc.tensor.transpose(out=tpf[:100, :msz],
                                                in_=sb2[:msz, :100],
                                                identity=ident_f[:msz, :msz])
                            nc.scalar.copy(out=rows[:, m0:m0 + msz],
                                           in_=tpf[:100, :msz])
                        nc.sync.dma_start(out=rel2bf[r0:r0 + 100, 0:D],
                                          in_=rows[:, 0:D])

                # gather ring state
                xg_bufs = {}
                rel_bufs = {}

                def get_xg(r, grp):
                    key = (r, grp)
                    if key not in xg_bufs:
                        t = xgp.tile([P, GT * DP], bf16, tag=f"xg{r}")
                        base = r * REGSZ
                        nrows = min(REGSZ, N - base)
                        nc.gpsimd.dma_gather(
                            out_ap=t[:].rearrange("p (k w) -> p k w", w=DP),
                            in_ap=xtable[base:base + nrows, :],
                            idxs_ap=xidx_t[r][:, grp * GT * 8:(grp + 1) * GT * 8],
                            num_idxs=GT * P, num_idxs_reg=GT * P,
                            elem_size=DP, single_packet=False)
                        xg_bufs[key] = t
                    return xg_bufs[key]

                def get_rel(grp):
                    if grp not in rel_bufs:
                        t = rgp.tile([P, GT * DP], bf16, tag="relg")
                        nc.gpsimd.dma_gather(
                            out_ap=t[:].rearrange("p (k w) -> p k w", w=DP),
                            in_ap=reltable[:, :],
                            idxs_ap=etidx_t[:, grp * GT * 8:(grp + 1) * GT * 8],
                            num_idxs=GT * P, num_idxs_reg=GT * P,
                            elem_size=DP, single_packet=False)
                        rel_bufs[grp] = t
                    return rel_bufs[grp]

                # ---------------- main loop ----------------
                tg = 0  # global tile index in G
                for w in range(NW):
                    aggs = []
                    for d in range(2):
                        aggA = pagg.tile([P, DP], f32, tag="aggA")
                        aggB = pagg.tile([72, DP], f32, tag="aggB")
                        ntile = int(T[d, w].sum())
                        k = 0
                        for r in range(NREG):
                            for _ in range(int(T[d, w, r])):
                                xg = get_xg(r, int(tile_xidx[tg]) // GT)
                                xs = (int(tile_xidx[tg]) % GT) * DP
                                rl = get_rel(tg // GT)
                                rs = (tg % GT) * DP
                                msg = mp.tile([P, DP], bf16, tag="msg")
                                nc.vector.tensor_tensor(
                                    out=msg[:], in0=xg[:, xs:xs + DP],
                                    in1=rl[:, rs:rs + DP],
                                    op=mybir.AluOpType.mult)
                                St = sp.tile([P, DP], bf16, tag="S")
                                nc.vector.tensor_scalar(
                                    out=St[:], in0=iota[:],
                                    scalar1=dstl_t[:, tg:tg + 1],
                                    scalar2=norm_t[:, tg:tg + 1],
                                    op0=mybir.AluOpType.is_equal,
                                    op1=mybir.AluOpType.mult)
                                nc.tensor.matmul(out=aggA[:], lhsT=msg[:, 0:128],
                                                 rhs=St[:], start=(k == 0),
                                                 stop=(k == ntile - 1))
                                nc.tensor.matmul(out=aggB[:], lhsT=msg[:, 128:D],
                                                 rhs=St[:], start=(k == 0),
                                                 stop=(k == ntile - 1))
                                k += 1
                                tg += 1
                        aggs.append((aggA, aggB))

                    # ---- window epilogue ----
                    # copy agg PSUM -> SBUF f32r
                    asb = []
                    for d in range(2):
                        cA = stg.tile([P, DP], f32r, tag="asbA")
                        cB = stg.tile([72, DP], f32r, tag="asbB")
                        nc.scalar.copy(out=cA[:], in_=aggs[d][0][:])
                        nc.scalar.copy(out=cB[:], in_=aggs[d][1][:])
                        asb.append((cA, cB))
                    # loop term: xT * loop_rel
                    lA = stg.tile([P, DP], f32r, tag="loopA")
                    lB = stg.tile([72, DP], f32r, tag="loopB")
                    for h in range(2):
                        xw = stg.tile([P, DP], bf16, tag="xw")
                        r0 = w * WIN + h * P
                        nc.sync.dma_start(out=xw[:], in_=xown[r0:r0 + P, :])
                        for (c0, csz), dstt in zip(KCH, (lA, lB)):
                            tp = ptp.tile([P, P], bf16, tag="ptp")
                            nc.tensor.transpose(out=tp[:csz, :],
                                                in_=xw[:, c0:c0 + csz],
                                                identity=ident_bf[:])
                            nc.scalar.mul(out=dstt[:csz, h * P:(h + 1) * P],
                                          in_=tp[:csz, :],
                                          mul=LR[l][0 if c0 == 0 else 1][:csz, :1])
                    # GEMMs: res = agg_in@Win + agg_out@Wout + loop@Wloop
                    terms = [(W[wi], asb[0]), (W[wo], asb[1]), (W[wl], (lA, lB))]
                    resP = []
                    for (m0, msz) in KCH:
                        op = pres.tile([P, DP], f32,
                                       tag=("presA" if m0 == 0 else "presB"))
                        first = True
                        for (Wt, rhs) in terms:
                            for ki, (k0, ksz) in enumerate(KCH):
                                nc.tensor.matmul(
                                    out=op[:msz, :],
                                    lhsT=Wt[ki][:, m0:m0 + msz],
                                    rhs=rhs[ki][:ksz, :],
                                    start=first,
                                    stop=(Wt is terms[2][0] and ki == 1))
                                first = False
                        resP.append(op)
                    # stash + stats
                    st = stats[l]
                    scr = stg.tile([P, DP], f32, tag="scr")
                    for (m0, msz), op, sname, qname, sd in zip(
                            KCH, resP, ("sA", "sB"), ("qA", "qB"),
                            (stashA, stashB)):
                        cpy = stg.tile([P, DP], f32, tag=f"stash{m0}")
                        nc.scalar.activation(
                            out=cpy[:msz, :], in_=op[:msz, :],
                            func=mybir.ActivationFunctionType.Copy,
                            accum_out=st[sname][:msz, w:w + 1])
                        nc.scalar.activation(
                            out=scr[:msz, :], in_=op[:msz, :],
                            func=mybir.ActivationFunctionType.Square,
                            accum_out=st[qname][:msz, w:w + 1])
                        nc.sync.dma_start(out=sd[w, 0:msz, :], in_=cpy[:msz, :])

                # ---------------- BN reduce + AllReduce ----------------
                st = stats[l]
                red = {}
                for nm, csz in [("sA", P), ("sB", 72), ("qA", P), ("qB", 72)]:
                    rt = cp.tile([P, 1], f32, tag=f"red{nm}{l}")
                    nc.vector.reduce_sum(out=rt[:csz, :1], in_=st[nm][:csz, :],
                                         axis=mybir.AxisListType.X)
                    red[nm] = rt
                nc.sync.dma_start(out=bn_in[l][0:128, 0:1], in_=red["sA"][:, :1])
                nc.sync.dma_start(out=bn_in[l][128:200, 0:1], in_=red["sB"][:72, :1])
                nc.sync.dma_start(out=bn_in[l][0:128, 1:2], in_=red["qA"][:, :1])
                nc.sync.dma_start(out=bn_in[l][128:200, 1:2], in_=red["qB"][:72, :1])
                nc.gpsimd.collective_compute(
                    "AllReduce", mybir.AluOpType.add, replica_groups=rg,
                    ins=[bn_in[l][:, :]], outs=[bn_out[l][:, :]])
                bn = {}
                for nm, (o0, csz, col) in {
                        "sA": (0, P, 0), "sB": (128, 72, 0),
                        "qA": (0, P, 1), "qB": (128, 72, 1)}.items():
                    rt = cp.tile([P, 1], f32, tag=f"bn{nm}{l}")
                    nc.sync.dma_start(out=rt[:csz, :1],
                                      in_=bn_out[l][o0:o0 + csz, col:col + 1])
                    bn[nm] = rt
                inv = {}
                nbias = {}
                for ch, csz in [("A", P), ("B", 72)]:
                    s_, q_ = bn["s" + ch], bn["q" + ch]
                    mu = cp.tile([P, 1], f32, tag=f"mu{ch}{l}")
                    nc.vector.tensor_scalar(out=mu[:csz], in0=s_[:csz],
                                            scalar1=1.0 / N, scalar2=None,
                                            op0=mybir.AluOpType.mult)
                    msq = cp.tile([P, 1], f32, tag=f"msq{ch}{l}")
                    nc.vector.tensor_scalar(out=msq[:csz], in0=q_[:csz],
                                            scalar1=1.0 / N, scalar2=None,
                                            op0=mybir.AluOpType.mult)
                    mu2 = cp.tile([P, 1], f32, tag=f"mu2{ch}{l}")
                    nc.vector.tensor_tensor(out=mu2[:csz], in0=mu[:csz],
                                            in1=mu[:csz],
                                            op=mybir.AluOpType.mult)
                    var = cp.tile([P, 1], f32, tag=f"var{ch}{l}")
                    nc.vector.tensor_tensor(out=var[:csz], in0=msq[:csz],
                                            in1=mu2[:csz],
                                            op=mybir.AluOpType.subtract)
                    std = cp.tile([P, 1], f32, tag=f"std{ch}{l}")
                    nc.scalar.activation(out=std[:csz], in_=var[:csz],
                                         func=mybir.ActivationFunctionType.Sqrt,
                                         bias=eps_t[:csz, :1])
                    iv = cp.tile([P, 1], f32, tag=f"inv{ch}{l}")
                    nc.vector.reciprocal(out=iv[:csz], in_=std[:csz])
                    t1 = cp.tile([P, 1], f32, tag=f"t1{ch}{l}")
                    nc.vector.tensor_tensor(out=t1[:csz], in0=mu[:csz],
                                            in1=iv[:csz],
                                            op=mybir.AluOpType.mult)
                    nb = cp.tile([P, 1], f32, tag=f"nb{ch}{l}")
                    nc.vector.tensor_scalar(out=nb[:csz], in0=t1[:csz],
                                            scalar1=-1.0, scalar2=None,
                                            op0=mybir.AluOpType.mult)
                    inv[ch] = iv
                    nbias[ch] = nb

                # ---------------- normalize + tanh + transpose out ----------
                odt = bf16 if l == 0 else f32
                ident = ident_bf if l == 0 else ident_f
                for w in range(NW):
                    ldA = stg.tile([P, DP], f32, tag="ldA")
                    ldB = stg.tile([72, DP], f32, tag="ldB")
                    nc.sync.dma_start(out=ldA[:], in_=stashA[w, :, :])
                    nc.sync.dma_start(out=ldB[:72], in_=stashB[w, :, :])
                    xpA = stg.tile([P, DP], odt, tag="xpA")
                    xpB = stg.tile([72, DP], odt, tag="xpB")
                    nc.scalar.activation(out=xpA[:], in_=ldA[:],
                                         func=mybir.ActivationFunctionType.Tanh,
                                         scale=inv["A"][:, :1],
                                         bias=nbias["A"][:, :1])
                    nc.scalar.activation(out=xpB[:72], in_=ldB[:72],
                                         func=mybir.ActivationFunctionType.Tanh,
                                         scale=inv["B"][:72, :1],
                                         bias=nbias["B"][:72, :1])
                    for h in range(2):
                        rowbase = w * WIN + h * P
                        nv = min(P, NSH - rowbase)
                        if nv <= 0:
                            continue
                        rows = rwp.tile([P, D], odt, tag="rows")
                        tpA = ptp.tile([P, P], odt, tag="ptp")
                        nc.tensor.transpose(out=tpA[:],
                                            in_=xpA[:, h * P:(h + 1) * P],
                                            identity=ident[:])
                        nc.scalar.copy(out=rows[:, 0:128], in_=tpA[:])
                        tpB = ptp.tile([P, P], odt, tag="ptp")
                        nc.tensor.transpose(out=tpB[:, :72],
                                            in_=xpB[:72, h * P:(h + 1) * P],
                                            identity=ident[:72, :72])
                        nc.scalar.copy(out=rows[:, 128:200], in_=tpB[:, :72])
                        if l == 0:
                            nc.sync.dma_start(out=ag_in[rowbase:rowbase + nv, 0:D],
                                              in_=rows[:nv, :])
                        else:
                            nc.sync.dma_start(out=x2_d[rowbase:rowbase + nv, :],
                                              in_=rows[:nv, :])

                if l == 0:
                    nc.gpsimd.collective_compute(
                        "AllGather", mybir.AluOpType.bypass, replica_groups=rg,
                        ins=[ag_in[0:NSH, :]], outs=[x1full[:, :]])

    nc.compile()
    return nc


_CACHE = {}


def kernel(**inputs):
    ei = np.asarray(inputs["edge_index"])
    et = np.asarray(inputs["edge_type"])
    meta, cores = _prep(ei, et)

    key = "prog"
    if key not in _CACHE:
        _CACHE[key] = _build(meta)
    nc = _CACHE[key]

    x0 = np.asarray(inputs["kg_node_emb"], np.float32)
    x0bf = np.zeros((N, DP), BF)
    x0bf[:, :D] = x0.astype(BF)
    rel1 = np.asarray(inputs["init_rel"], np.float32)
    rel1bf = np.zeros((R2, DP), BF)
    rel1bf[:, :D] = rel1.astype(BF)

    shared = {
        "x0bf": x0bf,
        "rel1bf": rel1bf,
        "rel1f": rel1,
        "win1": np.asarray(inputs["w_in1"], np.float32) / 3.0,
        "wout1": np.asarray(inputs["w_out1"], np.float32) / 3.0,
        "wloop1": np.asarray(inputs["w_loop1"], np.float32) / 3.0,
        "wrel1": np.asarray(inputs["w_rel1"], np.float32),
        "win2": np.asarray(inputs["w_in2"], np.float32) / 3.0,
        "wout2": np.asarray(inputs["w_out2"], np.float32) / 3.0,
        "wloop2": np.asarray(inputs["w_loop2"], np.float32) / 3.0,
        "lrel1": np.asarray(inputs["loop_rel1"], np.float32).reshape(D, 1),
        "lrel2": np.asarray(inputs["loop_rel2"], np.float32).reshape(D, 1),
    }
    in_maps = []
    for c in range(NCORE):
        m = dict(shared)
        x0own = np.zeros((NSH_PAD, DP), BF)
        x0own[:NSH] = x0bf[c * NSH:(c + 1) * NSH]
        m["x0own"] = x0own
        cd = cores[c]
        for r in range(NREG):
            m[f"xidx{r}"] = cd["xidx"][r]
        m["etidx"] = cd["etidx"]
        m["dstl"] = cd["dstl"]
        m["norm"] = cd["norm"]
        in_maps.append(m)

    trace = os.environ.get("COMPGCN_TRACE", "0") == "1"
    res = run_bass_kernel_spmd(nc, in_maps, core_ids=list(range(NCORE)),
                               trace=trace)
    if trace and res.exec_time_ns is not None:
        print(f"HW exec time: {res.exec_time_ns} ns")
        kernel.last_exec_ns = res.exec_time_ns

    x2 = np.concatenate([np.asarray(res.results[c]["x2"], np.float32)
                         for c in range(NCORE)], axis=0)
    org = np.asarray(inputs["org"]).astype(np.int64)
    return (x2[org], x2)


kernel.last_exec_ns = None


# revision 6
# speedup vs baseline: 1.3389x; 1.3389x over previous
"""CompGCN 2-layer forward on 8 Trainium2 NeuronCores (Bass/Tile).

Strategy (dst-sharded):
- Core c owns destination nodes [c*12500, (c+1)*12500), split into 49 windows of 256.
- Edges bucketed by (window, direction, src-region); scatter-add done as
  one-hot matmuls on TensorE: aggT[feat, node] += msg[edge, feat].T @ S[edge, node]
  with S[e, n] = (dstl[e] == n) * norm[e] built on DVE (bf16).
- Per-edge GEMM moved after aggregation (linearity): res = agg @ W.
- x / rel rows gathered with dma_gather (int16 idx, 4 src regions for x).
- BN+tanh fused into one ScalarE activation per tile; BN stats via tiny AllReduce.
- One AllGather of the layer-1 output shard; final unshard on host.
"""
import os
import numpy as np
import ml_dtypes

import concourse.bacc as bacc
import concourse.bass as bass
import concourse.mybir as mybir
import concourse.tile as tile
from concourse.bass_utils import run_bass_kernel_spmd
from concourse.masks import make_identity

N = 100000
D = 200
DP = 256
R2 = 400
E = 400000
NCORE = 8
NSH = 12500
WIN = 256
NW = 49            # ceil(12500/256)
NSH_PAD = NW * WIN  # 12544
NREG = 4
REGSZ = 25600
GT = 8             # tiles per gather call
P = 128
BN_EPS = 1e-5
BF = ml_dtypes.bfloat16

f32 = mybir.dt.float32
f32r = mybir.dt.float32r
bf16 = mybir.dt.bfloat16
i16 = mybir.dt.int16


def _wrap16(idx):
    """logical gather position i -> [i%16, i//16], replicated to 8 Q7 stripes."""
    a = idx.reshape(-1, 16).T.astype(np.int16)
    return np.ascontiguousarray(np.tile(a, (8, 1)))


def _slotmajor(a, dtype):
    """slot i -> [i%128, i//128]"""
    return np.ascontiguousarray(a.reshape(-1, P).T.astype(dtype))


def _prep(edge_index, edge_type):
    """Build uniform-across-cores tile structure + per-core slot arrays."""
    dirs = []
    for d in range(2):
        sl = slice(0, E) if d == 0 else slice(E, 2 * E)
        src = edge_index[0, sl].astype(np.int64)
        dst = edge_index[1, sl].astype(np.int64)
        et = edge_type[sl].astype(np.int64)
        deg = np.bincount(src, minlength=N).astype(np.float64)
        dinv = np.where(deg > 0, deg ** -0.5, 0.0)
        norm = (dinv[src] * dinv[dst]).astype(np.float32)
        dirs.append((src, dst, et, norm))

    # counts[c, d, w, r]
    counts = np.zeros((NCORE, 2, NW, NREG), np.int64)
    per_dir_sorted = []
    for d, (src, dst, et, norm) in enumerate(dirs):
        c = dst // NSH
        w = (dst % NSH) // WIN
        r = src // REGSZ
        key = ((c * 2 + 0) * NW + w) * NREG + r  # d folded later; sort within dir
        seg = (c * NW + w) * NREG + r
        order = np.lexsort((dst, seg))
        counts[:, d, :, :] = np.bincount(seg, minlength=NCORE * NW * NREG).reshape(
            NCORE, NW, NREG)
        per_dir_sorted.append((order, seg[order]))

    T = np.ceil(counts.max(axis=0) / P).astype(np.int64)  # [2, NW, NREG]
    for w in range(NW):
        for d in range(2):
            if T[d, w].sum() == 0:
                T[d, w, 0] = 1

    # G order: (w, d, r); tile index bookkeeping
    TOTG = int(T.sum())
    TOTG_PAD = ((TOTG + GT - 1) // GT) * GT
    LENG = TOTG * P
    NT_r = [int(T[:, :, r].sum()) for r in range(NREG)]
    NT_r_pad = [((n + GT - 1) // GT) * GT for n in NT_r]

    # per-G-tile: region, x-stream index within region
    tile_region = np.zeros(TOTG, np.int64)
    tile_xidx = np.zeros(TOTG, np.int64)
    xctr = [0] * NREG
    t = 0
    for w in range(NW):
        for d in range(2):
            for r in range(NREG):
                for _ in range(int(T[d, w, r])):
                    tile_region[t] = r
                    tile_xidx[t] = xctr[r]
                    xctr[r] += 1
                    t += 1
    assert t == TOTG

    cores = []
    for c in range(NCORE):
        et_slot = np.zeros(LENG, np.int64)
        dstl_slot = np.zeros(LENG, np.float32)
        norm_slot = np.zeros(LENG, np.float32)
        src_slot = np.zeros(LENG, np.int64)
        pos = 0
        for w in range(NW):
            for d in range(2):
                src, dst, et, norm = dirs[d]
                order, segs = per_dir_sorted[d]
                for r in range(NREG):
                    nt = int(T[d, w, r])
                    if nt == 0:
                        continue
                    seg_id = (c * NW + w) * NREG + r
                    lo = np.searchsorted(segs, seg_id, "left")
                    hi = np.searchsorted(segs, seg_id, "right")
                    idxs = order[lo:hi]
                    n = hi - lo
                    assert n <= nt * P
                    src_slot[pos:pos + n] = src[idxs]
                    et_slot[pos:pos + n] = et[idxs]
                    dstl_slot[pos:pos + n] = (dst[idxs] - c * NSH - w * WIN)
                    norm_slot[pos:pos + n] = norm[idxs]
                    pos += nt * P
        assert pos == LENG

        # x idx per region stream (G-order subsequence), padded to GT tiles
        xidx = []
        for r in range(NREG):
            sel = np.concatenate([
                src_slot[tg * P:(tg + 1) * P] for tg in range(TOTG)
                if tile_region[tg] == r]) if NT_r[r] else np.zeros(0, np.int64)
            arr = np.zeros(NT_r_pad[r] * P, np.int64)
            arr[:len(sel)] = sel - r * REGSZ
            # pad slots in real tiles have src_slot=0 -> idx -r*REGSZ < 0! fix:
            # pad slots must gather row 0 of the region (idx 0, killed by S=0).
            flat_norm = np.concatenate([
                norm_slot[tg * P:(tg + 1) * P] for tg in range(TOTG)
                if tile_region[tg] == r]) if NT_r[r] else np.zeros(0, np.float32)
            arr[:len(sel)][flat_norm == 0] = 0
            arr = np.clip(arr, 0, REGSZ - 1)
            xidx.append(_wrap16(arr))

        et_pad = np.zeros(TOTG_PAD * P, np.int64)
        et_pad[:LENG] = et_slot
        S = np.zeros((P, TOTG_PAD * DP), BF)
        ii = np.arange(LENG)
        S[ii % P, (ii // P) * DP + dstl_slot.astype(np.int64)] = norm_slot
        cores.append(dict(
            xidx=xidx,
            etidx=_wrap16(et_pad),
            smat=S,
        ))

    meta = dict(T=T, TOTG=TOTG, TOTG_PAD=TOTG_PAD, NT_r=NT_r, NT_r_pad=NT_r_pad,
                tile_region=tile_region, tile_xidx=tile_xidx)
    return meta, cores


def _build(meta):
    T = meta["T"]
    TOTG = meta["TOTG"]
    TOTG_PAD = meta["TOTG_PAD"]
    NT_r_pad = meta["NT_r_pad"]
    tile_region = meta["tile_region"]
    tile_xidx = meta["tile_xidx"]

    nc = bacc.Bacc("TRN2", target_bir_lowering=False, debug=False,
                   num_devices=NCORE, num_swdge_queues=4)

    # ---- inputs ----
    x0bf_d = nc.dram_tensor("x0bf", [N, DP], bf16, kind="ExternalInput")
    x0own_d = nc.dram_tensor("x0own", [NSH_PAD, DP], bf16, kind="ExternalInput")
    rel1bf_d = nc.dram_tensor("rel1bf", [R2, DP], bf16, kind="ExternalInput")
    rel1f_d = nc.dram_tensor("rel1f", [R2, D], f32, kind="ExternalInput")
    w_d = {}
    for name in ["win1", "wout1", "wloop1", "wrel1", "win2", "wout2", "wloop2"]:
        w_d[name] = nc.dram_tensor(name, [D, D], f32r, kind="ExternalInput")
    lr1_d = nc.dram_tensor("lrel1", [D, 1], f32, kind="ExternalInput")
    lr2_d = nc.dram_tensor("lrel2", [D, 1], f32, kind="ExternalInput")
    xidx_d = [nc.dram_tensor(f"xidx{r}", [P, NT_r_pad[r] * 8], i16,
                             kind="ExternalInput") for r in range(NREG)]
    etidx_d = nc.dram_tensor("etidx", [P, TOTG_PAD * 8], i16, kind="ExternalInput")
    smat_d = nc.dram_tensor("smat", [P, TOTG_PAD * DP], bf16, kind="ExternalInput")

    x2_d = nc.dram_tensor("x2", [NSH, D], f32, kind="ExternalOutput")

    # ---- internal DRAM ----
    ag_in = nc.dram_tensor("ag_in", [NSH_PAD, DP], bf16)
    x1full = nc.dram_tensor("x1full", [N, DP], bf16, addr_space="Shared")
    rel2bf = nc.dram_tensor("rel2bf", [R2, DP], bf16)
    stashA = nc.dram_tensor("stashA", [NW, P, DP], f32)
    stashB = nc.dram_tensor("stashB", [NW, 72, DP], f32)
    bn_in = [nc.dram_tensor(f"bn_in{l}", [DP, 2], f32) for l in range(2)]
    bn_out = [nc.dram_tensor(f"bn_out{l}", [DP, 2], f32, addr_space="Shared")
              for l in range(2)]
    rg = [list(range(NCORE))]

    KCH = [(0, 128), (128, 72)]  # feat chunks (offset, size)

    with tile.TileContext(nc) as tc:
        with (
            tc.tile_pool(name="const", bufs=1) as cp,
            tc.tile_pool(name="xg", bufs=3) as xgp,
            tc.tile_pool(name="rg", bufs=3) as rgp,
            tc.tile_pool(name="sS", bufs=4) as sp,
            tc.tile_pool(name="msg", bufs=4) as mp,
            tc.tile_pool(name="stg", bufs=3) as stg,
            tc.tile_pool(name="rows", bufs=3) as rwp,
            tc.tile_pool(name="pagg", bufs=2, space="PSUM") as pagg,
            tc.tile_pool(name="pres", bufs=1, space="PSUM") as pres,
            tc.tile_pool(name="ptp", bufs=2, space="PSUM") as ptp,
        ):
            # ---------------- prelude ----------------
            zt_bf = cp.tile([P, DP], bf16)
            nc.vector.memset(zt_bf[:], 0.0)
            zt_f = cp.tile([P, DP], f32)
            nc.vector.memset(zt_f[:], 0.0)
            # zero ag_in (incl. pad rows + pad cols)
            for t0 in range(0, NSH_PAD, P):
                nv = min(P, NSH_PAD - t0)
                nc.sync.dma_start(out=ag_in[t0:t0 + nv, :], in_=zt_bf[:nv, :])
            for t0 in range(0, R2, P):
                nv = min(P, R2 - t0)
                nc.sync.dma_start(out=rel2bf[t0:t0 + nv, :], in_=zt_bf[:nv, :])
            for l in range(2):
                nc.sync.dma_start(out=bn_in[l][0:128, :], in_=zt_f[:, :2])
                nc.sync.dma_start(out=bn_in[l][128:256, :], in_=zt_f[:, :2])

            eps_t = cp.tile([P, 1], f32)
            nc.vector.memset(eps_t[:], BN_EPS)
            ident_bf = cp.tile([P, P], bf16)
            make_identity(nc, ident_bf[:])
            ident_f = cp.tile([P, P], f32)
            make_identity(nc, ident_f[:])

            W = {}
            for name, dd in w_d.items():
                a = cp.tile([P, D], f32r, tag=f"W{name}a")
                b = cp.tile([72, D], f32r, tag=f"W{name}b")
                nc.sync.dma_start(out=a[:], in_=dd[0:128, :])
                nc.sync.dma_start(out=b[:], in_=dd[128:200, :])
                W[name] = (a, b)
            LR = []
            for dd in [lr1_d, lr2_d]:
                a = cp.tile([P, 1], f32, tag="lra")
                b = cp.tile([72, 1], f32, tag="lrb")
                nc.sync.dma_start(out=a[:], in_=dd[0:128, :])
                nc.sync.dma_start(out=b[:], in_=dd[128:200, :])
                LR.append((a, b))

            xidx_t = []
            for r in range(NREG):
                it = cp.tile([P, NT_r_pad[r] * 8], i16, tag=f"xidx{r}")
                nc.sync.dma_start(out=it[:], in_=xidx_d[r][:, :])
                xidx_t.append(it)
            etidx_t = cp.tile([P, TOTG_PAD * 8], i16)
            nc.sync.dma_start(out=etidx_t[:], in_=etidx_d[:, :])

            # BN stats tiles
            stats = []
            for l in range(2):
                stats.append({
                    nm: cp.tile([csz, NW], f32, tag=f"{nm}{l}",
                                name=f"stat_{nm}{l}")
                    for nm, csz in [("sA", P), ("sB", 72),
                                    ("qA", P), ("qB", 72)]
                })

            # ---------------- per layer ----------------
            for l in range(2):
                xtable = x0bf_d if l == 0 else x1full
                reltable = rel1bf_d if l == 0 else rel2bf
                xown = x0own_d if l == 0 else ag_in
                wi, wo, wl = (("win1", "wout1", "wloop1") if l == 0
                              else ("win2", "wout2", "wloop2"))

                if l == 1:
                    # rel2 = init_rel @ w_rel1  (row-major bf16 into rel2bf)
                    for rc in range(4):
                        r0 = rc * 100
                        rl = stg.tile([100, D], f32, tag="r2in")
                        nc.sync.dma_start(out=rl[:], in_=rel1f_d[r0:r0 + 100, :])
                        rT = []
                        for (c0, csz) in KCH:
                            tp = ptp.tile([P, P], f32, tag="ptp")
                            nc.tensor.transpose(out=tp[:csz, :100],
                                                in_=rl[:, c0:c0 + csz],
                                                identity=ident_f[:100, :100])
                            sbT = stg.tile([P, 100], f32r, tag=f"r2T{c0}")
                            nc.scalar.copy(out=sbT[:csz, :], in_=tp[:csz, :100])
                            rT.append((sbT, csz))
                        outs = []
                        for (m0, msz) in KCH:
                            op = pres.tile([P, 100], f32, tag="presA")
                            for ki, ((k0, ksz), (rhs, _)) in enumerate(
                                    zip(KCH, rT)):
                                nc.tensor.matmul(
                                    out=op[:msz, :],
                                    lhsT=W["wrel1"][ki][:, m0:m0 + msz],
                                    rhs=rhs[:ksz, :],
                                    start=(ki == 0), stop=(ki == 1))
                            sb2 = stg.tile([P, 100], f32, tag=f"r2o{m0}")
                            nc.scalar.copy(out=sb2[:msz, :], in_=op[:msz, :])
                            outs.append((sb2, msz))
                        rows = rwp.tile([100, DP], bf16, tag="r2rows")
                        for (sb2, msz), (m0, _) in zip(outs, KCH):
                            tpf = ptp.tile([P, P], f32, tag="ptp")
                            nc.tensor.transpose(out=tpf[:100, :msz],
                                                in_=sb2[:msz, :100],
                                                identity=ident_f[:msz, :msz])
                            nc.scalar.copy(out=rows[:, m0:m0 + msz],
                                           in_=tpf[:100, :msz])
                        nc.sync.dma_start(out=rel2bf[r0:r0 + 100, 0:D],
                                          in_=rows[:, 0:D])

                # gather ring state
                xg_bufs = {}
                rel_bufs = {}
                smat_bufs = {}

                def get_xg(r, grp):
                    key = (r, grp)
                    if key not in xg_bufs:
                        t = xgp.tile([P, GT * DP], bf16, tag=f"xg{r}")
                        base = r * REGSZ
                        nrows = min(REGSZ, N - base)
                        nc.gpsimd.dma_gather(
                            out_ap=t[:].rearrange("p (k w) -> p k w", w=DP),
                            in_ap=xtable[base:base + nrows, :],
                            idxs_ap=xidx_t[r][:, grp * GT * 8:(grp + 1) * GT * 8],
                            num_idxs=GT * P, num_idxs_reg=GT * P,
                            elem_size=DP, single_packet=False,
                            queue_num=r)
                        xg_bufs[key] = t
                    return xg_bufs[key]

                def get_rel(grp):
                    if grp not in rel_bufs:
                        t = rgp.tile([P, GT * DP], bf16, tag="relg")
                        nc.gpsimd.dma_gather(
                            out_ap=t[:].rearrange("p (k w) -> p k w", w=DP),
                            in_ap=reltable[:, :],
                            idxs_ap=etidx_t[:, grp * GT * 8:(grp + 1) * GT * 8],
                            num_idxs=GT * P, num_idxs_reg=GT * P,
                            elem_size=DP, single_packet=False,
                            queue_num=grp % 4)
                        rel_bufs[grp] = t
                    return rel_bufs[grp]

                def get_smat(grp):
                    if grp not in smat_bufs:
                        t = rgp.tile([P, GT * DP], bf16, tag="smat")
                        nc.sync.dma_start(
                            out=t[:],
                            in_=smat_d[:, grp * GT * DP:(grp + 1) * GT * DP])
                        smat_bufs[grp] = t
                    return smat_bufs[grp]

                # ---------------- main loop ----------------
                tg = 0  # global tile index in G
                for w in range(NW):
                    aggs = []
                    for d in range(2):
                        aggA = pagg.tile([P, DP], f32, tag="aggA")
                        aggB = pagg.tile([72, DP], f32, tag="aggB")
                        ntile = int(T[d, w].sum())
                        k = 0
                        for r in range(NREG):
                            for _ in range(int(T[d, w, r])):
                                xg = get_xg(r, int(tile_xidx[tg]) // GT)
                                xs = (int(tile_xidx[tg]) % GT) * DP
                                rl = get_rel(tg // GT)
                                rs = (tg % GT) * DP
                                msg = mp.tile([P, DP], bf16, tag="msg")
                                nc.vector.tensor_tensor(
                                    out=msg[:], in0=xg[:, xs:xs + DP],
                                    in1=rl[:, rs:rs + DP],
                                    op=mybir.AluOpType.mult)
                                Sg = get_smat(tg // GT)
                                Ss = (tg % GT) * DP
                                nc.tensor.matmul(out=aggA[:], lhsT=msg[:, 0:128],
                                                 rhs=Sg[:, Ss:Ss + DP],
                                                 start=(k == 0),
                                                 stop=(k == ntile - 1))
                                nc.tensor.matmul(out=aggB[:], lhsT=msg[:, 128:D],
                                                 rhs=Sg[:, Ss:Ss + DP],
                                                 start=(k == 0),
                                                 stop=(k == ntile - 1))
                                k += 1
                                tg += 1
                        aggs.append((aggA, aggB))

                    # ---- window epilogue ----
                    # copy agg PSUM -> SBUF f32r
                    asb = []
                    for d in range(2):
                        cA = stg.tile([P, DP], f32r, tag="asbA")
                        cB = stg.tile([72, DP], f32r, tag="asbB")
                        nc.scalar.copy(out=cA[:], in_=aggs[d][0][:])
                        nc.scalar.copy(out=cB[:], in_=aggs[d][1][:])
                        asb.append((cA, cB))
                    # loop term: xT * loop_rel
                    lA = stg.tile([P, DP], f32r, tag="loopA")
                    lB = stg.tile([72, DP], f32r, tag="loopB")
                    for h in range(2):
                        xw = stg.tile([P, DP], bf16, tag="xw")
                        r0 = w * WIN + h * P
                        nc.sync.dma_start(out=xw[:], in_=xown[r0:r0 + P, :])
                        for (c0, csz), dstt in zip(KCH, (lA, lB)):
                            tp = ptp.tile([P, P], bf16, tag="ptp")
                            nc.tensor.transpose(out=tp[:csz, :],
                                                in_=xw[:, c0:c0 + csz],
                                                identity=ident_bf[:])
                            nc.scalar.mul(out=dstt[:csz, h * P:(h + 1) * P],
                                          in_=tp[:csz, :],
                                          mul=LR[l][0 if c0 == 0 else 1][:csz, :1])
                    # GEMMs: res = agg_in@Win + agg_out@Wout + loop@Wloop
                    terms = [(W[wi], asb[0]), (W[wo], asb[1]), (W[wl], (lA, lB))]
                    resP = []
                    for (m0, msz) in KCH:
                        op = pres.tile([P, DP], f32,
                                       tag=("presA" if m0 == 0 else "presB"))
                        first = True
                        for (Wt, rhs) in terms:
                            for ki, (k0, ksz) in enumerate(KCH):
                                nc.tensor.matmul(
                                    out=op[:msz, :],
                                    lhsT=Wt[ki][:, m0:m0 + msz],
                                    rhs=rhs[ki][:ksz, :],
                                    start=first,
                                    stop=(Wt is terms[2][0] and ki == 1))
                                first = False
                        resP.append(op)
                    # stash + stats
                    st = stats[l]
                    scr = stg.tile([P, DP], f32, tag="scr")
                    for (m0, msz), op, sname, qname, sd in zip(
                            KCH, resP, ("sA", "sB"), ("qA", "qB"),
                            (stashA, stashB)):
                        cpy = stg.tile([P, DP], f32, tag=f"stash{m0}")
                        nc.scalar.activation(
                            out=cpy[:msz, :], in_=op[:msz, :],
                            func=mybir.ActivationFunctionType.Copy,
                            accum_out=st[sname][:msz, w:w + 1])
                        nc.scalar.activation(
                            out=scr[:msz, :], in_=op[:msz, :],
                            func=mybir.ActivationFunctionType.Square,
                            accum_out=st[qname][:msz, w:w + 1])
                        nc.sync.dma_start(out=sd[w, 0:msz, :], in_=cpy[:msz, :])

                # ---------------- BN reduce + AllReduce ----------------
                st = stats[l]
                red = {}
                for nm, csz in [("sA", P), ("sB", 72), ("qA", P), ("qB", 72)]:
                    rt = cp.tile([P, 1], f32, tag=f"red{nm}{l}")
                    nc.vector.reduce_sum(out=rt[:csz, :1], in_=st[nm][:csz, :],
                                         axis=mybir.AxisListType.X)
                    red[nm] = rt
                nc.sync.dma_start(out=bn_in[l][0:128, 0:1], in_=red["sA"][:, :1])
                nc.sync.dma_start(out=bn_in[l][128:200, 0:1], in_=red["sB"][:72, :1])
                nc.sync.dma_start(out=bn_in[l][0:128, 1:2], in_=red["qA"][:, :1])
                nc.sync.dma_start(out=bn_in[l][128:200, 1:2], in_=red["qB"][:72, :1])
                nc.gpsimd.collective_compute(
                    "AllReduce", mybir.AluOpType.add, replica_groups=rg,
                    ins=[bn_in[l][:, :]], outs=[bn_out[l][:, :]])
                bn = {}
                for nm, (o0, csz, col) in {
                        "sA": (0, P, 0), "sB": (128, 72, 0),
                        "qA": (0, P, 1), "qB": (128, 72, 1)}.items():
                    rt = cp.tile([P, 1], f32, tag=f"bn{nm}{l}")
                    nc.sync.dma_start(out=rt[:csz, :1],
                                      in_=bn_out[l][o0:o0 + csz, col:col + 1])
                    bn[nm] = rt
                inv = {}
                nbias = {}
                for ch, csz in [("A", P), ("B", 72)]:
                    s_, q_ = bn["s" + ch], bn["q" + ch]
                    mu = cp.tile([P, 1], f32, tag=f"mu{ch}{l}")
                    nc.vector.tensor_scalar(out=mu[:csz], in0=s_[:csz],
                                            scalar1=1.0 / N, scalar2=None,
                                            op0=mybir.AluOpType.mult)
                    msq = cp.tile([P, 1], f32, tag=f"msq{ch}{l}")
                    nc.vector.tensor_scalar(out=msq[:csz], in0=q_[:csz],
                                            scalar1=1.0 / N, scalar2=None,
                                            op0=mybir.AluOpType.mult)
                    mu2 = cp.tile([P, 1], f32, tag=f"mu2{ch}{l}")
                    nc.vector.tensor_tensor(out=mu2[:csz], in0=mu[:csz],
                                            in1=mu[:csz],
                                            op=mybir.AluOpType.mult)
                    var = cp.tile([P, 1], f32, tag=f"var{ch}{l}")
                    nc.vector.tensor_tensor(out=var[:csz], in0=msq[:csz],
                                            in1=mu2[:csz],
                                            op=mybir.AluOpType.subtract)
                    std = cp.tile([P, 1], f32, tag=f"std{ch}{l}")
                    nc.scalar.activation(out=std[:csz], in_=var[:csz],
                                         func=mybir.ActivationFunctionType.Sqrt,
                                         bias=eps_t[:csz, :1])
                    iv = cp.tile([P, 1], f32, tag=f"inv{ch}{l}")
                    nc.vector.reciprocal(out=iv[:csz], in_=std[:csz])
                    t1 = cp.tile([P, 1], f32, tag=f"t1{ch}{l}")
                    nc.vector.tensor_tensor(out=t1[:csz], in0=mu[:csz],
                                            in1=iv[:csz],
                                            op=mybir.AluOpType.mult)
                    nb = cp.tile([P, 1], f32, tag=f"nb{ch}{l}")
                    nc.vector.tensor_scalar(out=nb[:csz], in0=t1[:csz],
                                            scalar1=-1.0, scalar2=None,
                                            op0=mybir.AluOpType.mult)
                    inv[ch] = iv
                    nbias[ch] = nb

                # ---------------- normalize + tanh + transpose out ----------
                odt = bf16 if l == 0 else f32
                ident = ident_bf if l == 0 else ident_f
                for w in range(NW):
                    ldA = stg.tile([P, DP], f32, tag="ldA")
                    ldB = stg.tile([72, DP], f32, tag="ldB")
                    nc.sync.dma_start(out=ldA[:], in_=stashA[w, :, :])
                    nc.sync.dma_start(out=ldB[:72], in_=stashB[w, :, :])
                    xpA = stg.tile([P, DP], odt, tag="xpA")
                    xpB = stg.tile([72, DP], odt, tag="xpB")
                    nc.scalar.activation(out=xpA[:], in_=ldA[:],
                                         func=mybir.ActivationFunctionType.Tanh,
                                         scale=inv["A"][:, :1],
                                         bias=nbias["A"][:, :1])
                    nc.scalar.activation(out=xpB[:72], in_=ldB[:72],
                                         func=mybir.ActivationFunctionType.Tanh,
                                         scale=inv["B"][:72, :1],
                                         bias=nbias["B"][:72, :1])
                    for h in range(2):
                        rowbase = w * WIN + h * P
                        nv = min(P, NSH - rowbase)
                        if nv <= 0:
                            continue
                        rows = rwp.tile([P, D], odt, tag="rows")
                        tpA = ptp.tile([P, P], odt, tag="ptp")
                        nc.tensor.transpose(out=tpA[:],
                                            in_=xpA[:, h * P:(h + 1) * P],
                                            identity=ident[:])
                        nc.scalar.copy(out=rows[:, 0:128], in_=tpA[:])
                        tpB = ptp.tile([P, P], odt, tag="ptp")
                        nc.tensor.transpose(out=tpB[:, :72],
                                            in_=xpB[:72, h * P:(h + 1) * P],
                                            identity=ident[:72, :72])
                        nc.scalar.copy(out=rows[:, 128:200], in_=tpB[:, :72])
                        if l == 0:
                            nc.sync.dma_start(out=ag_in[rowbase:rowbase + nv, 0:D],
                                              in_=rows[:nv, :])
                        else:
                            nc.sync.dma_start(out=x2_d[rowbase:rowbase + nv, :],
                                              in_=rows[:nv, :])

                if l == 0:
                    nc.gpsimd.collective_compute(
                        "AllGather", mybir.AluOpType.bypass, replica_groups=rg,
                        ins=[ag_in[0:NSH, :]], outs=[x1full[:, :]])

    nc.compile()
    return nc


_CACHE = {}


def kernel(**inputs):
    ei = np.asarray(inputs["edge_index"])
    et = np.asarray(inputs["edge_type"])
    meta, cores = _prep(ei, et)

    key = "prog"
    if key not in _CACHE:
        _CACHE[key] = _build(meta)
    nc = _CACHE[key]

    x0 = np.asarray(inputs["kg_node_emb"], np.float32)
    x0bf = np.zeros((N, DP), BF)
    x0bf[:, :D] = x0.astype(BF)
    rel1 = np.asarray(inputs["init_rel"], np.float32)
    rel1bf = np.zeros((R2, DP), BF)
    rel1bf[:, :D] = rel1.astype(BF)

    shared = {
        "x0bf": x0bf,
        "rel1bf": rel1bf,
        "rel1f": rel1,
        "win1": np.asarray(inputs["w_in1"], np.float32) / 3.0,
        "wout1": np.asarray(inputs["w_out1"], np.float32) / 3.0,
        "wloop1": np.asarray(inputs["w_loop1"], np.float32) / 3.0,
        "wrel1": np.asarray(inputs["w_rel1"], np.float32),
        "win2": np.asarray(inputs["w_in2"], np.float32) / 3.0,
        "wout2": np.asarray(inputs["w_out2"], np.float32) / 3.0,
        "wloop2": np.asarray(inputs["w_loop2"], np.float32) / 3.0,
        "lrel1": np.asarray(inputs["loop_rel1"], np.float32).reshape(D, 1),
        "lrel2": np.asarray(inputs["loop_rel2"], np.float32).reshape(D, 1),
    }
    in_maps = []
    for c in range(NCORE):
        m = dict(shared)
        x0own = np.zeros((NSH_PAD, DP), BF)
        x0own[:NSH] = x0bf[c * NSH:(c + 1) * NSH]
        m["x0own"] = x0own
        cd = cores[c]
        for r in range(NREG):
            m[f"xidx{r}"] = cd["xidx"][r]
        m["etidx"] = cd["etidx"]
        m["smat"] = cd["smat"]
        in_maps.append(m)

    trace = os.environ.get("COMPGCN_TRACE", "0") == "1"
    res = run_bass_kernel_spmd(nc, in_maps, core_ids=list(range(NCORE)),
                               trace=trace)
    if trace and res.exec_time_ns is not None:
        print(f"HW exec time: {res.exec_time_ns} ns")
        kernel.last_exec_ns = res.exec_time_ns

    x2 = np.concatenate([np.asarray(res.results[c]["x2"], np.float32)
                         for c in range(NCORE)], axis=0)
    org = np.asarray(inputs["org"]).astype(np.int64)
    return (x2[org], x2)


kernel.last_exec_ns = None


# revision 7
# speedup vs baseline: 1.8752x; 1.4005x over previous
"""CompGCN 2-layer forward on 8 Trainium2 NeuronCores (Bass/Tile).

Strategy (dst-sharded):
- Core c owns destination nodes [c*12500, (c+1)*12500), split into 49 windows of 256.
- Edges bucketed by (window, direction, src-region); scatter-add done as
  one-hot matmuls on TensorE: aggT[feat, node] += msg[edge, feat].T @ S[edge, node]
  with S[e, n] = (dstl[e] == n) * norm[e] built on DVE (bf16).
- Per-edge GEMM moved after aggregation (linearity): res = agg @ W.
- x / rel rows gathered with dma_gather (int16 idx, 4 src regions for x).
- BN+tanh fused into one ScalarE activation per tile; BN stats via tiny AllReduce.
- One AllGather of the layer-1 output shard; final unshard on host.
"""
import os
import numpy as np
import ml_dtypes

import concourse.bacc as bacc
import concourse.bass as bass
import concourse.mybir as mybir
import concourse.tile as tile
from concourse.bass_utils import run_bass_kernel_spmd
from concourse.masks import make_identity

N = 100000
D = 200
DP = 256
R2 = 400
E = 400000
NCORE = 8
NSH = 12500
WIN = 256
NW = 49            # ceil(12500/256)
NSH_PAD = NW * WIN  # 12544
NREG = 4
REGSZ = 25600
GT = 8             # tiles per gather call
P = 128
BN_EPS = 1e-5
BF = ml_dtypes.bfloat16

f32 = mybir.dt.float32
f32r = mybir.dt.float32r
bf16 = mybir.dt.bfloat16
i16 = mybir.dt.int16


def _wrap16(idx):
    """logical gather position i -> [i%16, i//16], replicated to 8 Q7 stripes."""
    a = idx.reshape(-1, 16).T.astype(np.int16)
    return np.ascontiguousarray(np.tile(a, (8, 1)))


def _slotmajor(a, dtype):
    """slot i -> [i%128, i//128]"""
    return np.ascontiguousarray(a.reshape(-1, P).T.astype(dtype))


def _prep(edge_index, edge_type):
    """Build uniform-across-cores tile structure + per-core slot arrays."""
    dirs = []
    for d in range(2):
        sl = slice(0, E) if d == 0 else slice(E, 2 * E)
        src = edge_index[0, sl].astype(np.int64)
        dst = edge_index[1, sl].astype(np.int64)
        et = edge_type[sl].astype(np.int64)
        deg = np.bincount(src, minlength=N).astype(np.float64)
        dinv = np.where(deg > 0, deg ** -0.5, 0.0)
        norm = (dinv[src] * dinv[dst]).astype(np.float32)
        dirs.append((src, dst, et, norm))

    # counts[c, d, w, r]
    counts = np.zeros((NCORE, 2, NW, NREG), np.int64)
    per_dir_sorted = []
    for d, (src, dst, et, norm) in enumerate(dirs):
        c = dst // NSH
        w = (dst % NSH) // WIN
        r = src // REGSZ
        key = ((c * 2 + 0) * NW + w) * NREG + r  # d folded later; sort within dir
        seg = (c * NW + w) * NREG + r
        order = np.lexsort((dst, seg))
        counts[:, d, :, :] = np.bincount(seg, minlength=NCORE * NW * NREG).reshape(
            NCORE, NW, NREG)
        per_dir_sorted.append((order, seg[order]))

    T = np.ceil(counts.max(axis=0) / P).astype(np.int64)  # [2, NW, NREG]
    for w in range(NW):
        for d in range(2):
            if T[d, w].sum() == 0:
                T[d, w, 0] = 1

    # G order: (w, d, r); tile index bookkeeping
    TOTG = int(T.sum())
    TOTG_PAD = ((TOTG + GT - 1) // GT) * GT
    LENG = TOTG * P
    NT_r = [int(T[:, :, r].sum()) for r in range(NREG)]
    NT_r_pad = [((n + GT - 1) // GT) * GT for n in NT_r]

    # per-G-tile: region, x-stream index within region
    tile_region = np.zeros(TOTG, np.int64)
    tile_xidx = np.zeros(TOTG, np.int64)
    xctr = [0] * NREG
    t = 0
    for w in range(NW):
        for d in range(2):
            for r in range(NREG):
                for _ in range(int(T[d, w, r])):
                    tile_region[t] = r
                    tile_xidx[t] = xctr[r]
                    xctr[r] += 1
                    t += 1
    assert t == TOTG

    cores = []
    for c in range(NCORE):
        et_slot = np.zeros(LENG, np.int64)
        dstl_slot = np.zeros(LENG, np.float32)
        norm_slot = np.zeros(LENG, np.float32)
        src_slot = np.zeros(LENG, np.int64)
        pos = 0
        for w in range(NW):
            for d in range(2):
                src, dst, et, norm = dirs[d]
                order, segs = per_dir_sorted[d]
                for r in range(NREG):
                    nt = int(T[d, w, r])
                    if nt == 0:
                        continue
                    seg_id = (c * NW + w) * NREG + r
                    lo = np.searchsorted(segs, seg_id, "left")
                    hi = np.searchsorted(segs, seg_id, "right")
                    idxs = order[lo:hi]
                    n = hi - lo
                    assert n <= nt * P
                    src_slot[pos:pos + n] = src[idxs]
                    et_slot[pos:pos + n] = et[idxs]
                    dstl_slot[pos:pos + n] = (dst[idxs] - c * NSH - w * WIN)
                    norm_slot[pos:pos + n] = norm[idxs]
                    pos += nt * P
        assert pos == LENG

        # x idx per region stream (G-order subsequence), padded to GT tiles
        xidx = []
        for r in range(NREG):
            sel = np.concatenate([
                src_slot[tg * P:(tg + 1) * P] for tg in range(TOTG)
                if tile_region[tg] == r]) if NT_r[r] else np.zeros(0, np.int64)
            arr = np.zeros(NT_r_pad[r] * P, np.int64)
            arr[:len(sel)] = sel - r * REGSZ
            # pad slots in real tiles have src_slot=0 -> idx -r*REGSZ < 0! fix:
            # pad slots must gather row 0 of the region (idx 0, killed by S=0).
            flat_norm = np.concatenate([
                norm_slot[tg * P:(tg + 1) * P] for tg in range(TOTG)
                if tile_region[tg] == r]) if NT_r[r] else np.zeros(0, np.float32)
            arr[:len(sel)][flat_norm == 0] = 0
            arr = np.clip(arr, 0, REGSZ - 1)
            xidx.append(_wrap16(arr))

        et_pad = np.zeros(TOTG_PAD * P, np.int64)
        et_pad[:LENG] = et_slot
        S = np.zeros((P, TOTG_PAD * DP), BF)
        ii = np.arange(LENG)
        S[ii % P, (ii // P) * DP + dstl_slot.astype(np.int64)] = norm_slot
        cores.append(dict(
            xidx=xidx,
            etrows=et_pad,
            smat=S,
        ))

    meta = dict(T=T, TOTG=TOTG, TOTG_PAD=TOTG_PAD, NT_r=NT_r, NT_r_pad=NT_r_pad,
                tile_region=tile_region, tile_xidx=tile_xidx)
    return meta, cores


def _build(meta):
    T = meta["T"]
    TOTG = meta["TOTG"]
    TOTG_PAD = meta["TOTG_PAD"]
    NT_r_pad = meta["NT_r_pad"]
    tile_region = meta["tile_region"]
    tile_xidx = meta["tile_xidx"]

    nc = bacc.Bacc("TRN2", target_bir_lowering=False, debug=False,
                   num_devices=NCORE, num_swdge_queues=4)

    # ---- inputs ----
    x0bf_d = nc.dram_tensor("x0bf", [N, DP], bf16, kind="ExternalInput")
    x0own_d = nc.dram_tensor("x0own", [NSH_PAD, DP], bf16, kind="ExternalInput")
    relg_d = [nc.dram_tensor(f"relg{l}", [P, TOTG_PAD * DP], bf16,
                             kind="ExternalInput") for l in range(2)]
    w_d = {}
    for name in ["win1", "wout1", "wloop1", "win2", "wout2", "wloop2"]:
        w_d[name] = nc.dram_tensor(name, [D, D], f32r, kind="ExternalInput")
    lr1_d = nc.dram_tensor("lrel1", [D, 1], f32, kind="ExternalInput")
    lr2_d = nc.dram_tensor("lrel2", [D, 1], f32, kind="ExternalInput")
    xidx_d = [nc.dram_tensor(f"xidx{r}", [P, NT_r_pad[r] * 8], i16,
                             kind="ExternalInput") for r in range(NREG)]
    smat_d = nc.dram_tensor("smat", [P, TOTG_PAD * DP], bf16, kind="ExternalInput")

    x2_d = nc.dram_tensor("x2", [NSH, D], f32, kind="ExternalOutput")

    # ---- internal DRAM ----
    ag_in = nc.dram_tensor("ag_in", [NSH_PAD, DP], bf16)
    x1full = nc.dram_tensor("x1full", [N, DP], bf16, addr_space="Shared")
    stashA = nc.dram_tensor("stashA", [NW, P, DP], f32)
    stashB = nc.dram_tensor("stashB", [NW, 72, DP], f32)
    bn_in = [nc.dram_tensor(f"bn_in{l}", [DP, 2], f32) for l in range(2)]
    bn_out = [nc.dram_tensor(f"bn_out{l}", [DP, 2], f32, addr_space="Shared")
              for l in range(2)]
    rg = [list(range(NCORE))]

    KCH = [(0, 128), (128, 72)]  # feat chunks (offset, size)

    with tile.TileContext(nc) as tc:
        with (
            tc.tile_pool(name="const", bufs=1) as cp,
            tc.tile_pool(name="xg", bufs=4) as xgp,
            tc.tile_pool(name="rg", bufs=4) as rgp,
            tc.tile_pool(name="sS", bufs=4) as sp,
            tc.tile_pool(name="msg", bufs=6) as mp,
            tc.tile_pool(name="stg", bufs=3) as stg,
            tc.tile_pool(name="rows", bufs=3) as rwp,
            tc.tile_pool(name="pagg", bufs=2, space="PSUM") as pagg,
            tc.tile_pool(name="pres", bufs=1, space="PSUM") as pres,
            tc.tile_pool(name="ptp", bufs=2, space="PSUM") as ptp,
        ):
            # ---------------- prelude ----------------
            zt_bf = cp.tile([P, DP], bf16)
            nc.vector.memset(zt_bf[:], 0.0)
            zt_f = cp.tile([P, DP], f32)
            nc.vector.memset(zt_f[:], 0.0)
            # zero ag_in (incl. pad rows + pad cols)
            for t0 in range(0, NSH_PAD, P):
                nv = min(P, NSH_PAD - t0)
                nc.sync.dma_start(out=ag_in[t0:t0 + nv, :], in_=zt_bf[:nv, :])
            for l in range(2):
                nc.sync.dma_start(out=bn_in[l][0:128, :], in_=zt_f[:, :2])
                nc.sync.dma_start(out=bn_in[l][128:256, :], in_=zt_f[:, :2])

            eps_t = cp.tile([P, 1], f32)
            nc.vector.memset(eps_t[:], BN_EPS)
            ident_bf = cp.tile([P, P], bf16)
            make_identity(nc, ident_bf[:])
            ident_f = cp.tile([P, P], f32)
            make_identity(nc, ident_f[:])

            W = {}
            for name, dd in w_d.items():
                a = cp.tile([P, D], f32r, tag=f"W{name}a")
                b = cp.tile([72, D], f32r, tag=f"W{name}b")
                nc.sync.dma_start(out=a[:], in_=dd[0:128, :])
                nc.sync.dma_start(out=b[:], in_=dd[128:200, :])
                W[name] = (a, b)
            LR = []
            for dd in [lr1_d, lr2_d]:
                a = cp.tile([P, 1], f32, tag="lra")
                b = cp.tile([72, 1], f32, tag="lrb")
                nc.sync.dma_start(out=a[:], in_=dd[0:128, :])
                nc.sync.dma_start(out=b[:], in_=dd[128:200, :])
                LR.append((a, b))

            xidx_t = []
            for r in range(NREG):
                it = cp.tile([P, NT_r_pad[r] * 8], i16, tag=f"xidx{r}")
                nc.sync.dma_start(out=it[:], in_=xidx_d[r][:, :])
                xidx_t.append(it)

            # BN stats tiles
            stats = []
            for l in range(2):
                stats.append({
                    nm: cp.tile([csz, NW], f32, tag=f"{nm}{l}",
                                name=f"stat_{nm}{l}")
                    for nm, csz in [("sA", P), ("sB", 72),
                                    ("qA", P), ("qB", 72)]
                })

            # ---------------- per layer ----------------
            for l in range(2):
                xtable = x0bf_d if l == 0 else x1full
                xown = x0own_d if l == 0 else ag_in
                wi, wo, wl = (("win1", "wout1", "wloop1") if l == 0
                              else ("win2", "wout2", "wloop2"))

                # gather ring state
                xg_bufs = {}
                rel_bufs = {}
                smat_bufs = {}

                def get_xg(r, grp):
                    key = (r, grp)
                    if key not in xg_bufs:
                        t = xgp.tile([P, GT * DP], bf16, tag=f"xg{r}")
                        base = r * REGSZ
                        nrows = min(REGSZ, N - base)
                        nc.gpsimd.dma_gather(
                            out_ap=t[:].rearrange("p (k w) -> p k w", w=DP),
                            in_ap=xtable[base:base + nrows, :],
                            idxs_ap=xidx_t[r][:, grp * GT * 8:(grp + 1) * GT * 8],
                            num_idxs=GT * P, num_idxs_reg=GT * P,
                            elem_size=DP, single_packet=False,
                            queue_num=r)
                        xg_bufs[key] = t
                    return xg_bufs[key]

                def get_rel(grp):
                    if grp not in rel_bufs:
                        t = rgp.tile([P, GT * DP], bf16, tag="relg")
                        nc.sync.dma_start(
                            out=t[:],
                            in_=relg_d[l][:, grp * GT * DP:(grp + 1) * GT * DP])
                        rel_bufs[grp] = t
                    return rel_bufs[grp]

                def get_smat(grp):
                    if grp not in smat_bufs:
                        t = rgp.tile([P, GT * DP], bf16, tag="smat")
                        nc.sync.dma_start(
                            out=t[:],
                            in_=smat_d[:, grp * GT * DP:(grp + 1) * GT * DP])
                        smat_bufs[grp] = t
                    return smat_bufs[grp]

                # ---------------- main loop ----------------
                tg = 0  # global tile index in G
                for w in range(NW):
                    aggs = []
                    for d in range(2):
                        aggA = pagg.tile([P, DP], f32, tag="aggA")
                        aggB = pagg.tile([72, DP], f32, tag="aggB")
                        ntile = int(T[d, w].sum())
                        k = 0
                        for r in range(NREG):
                            for _ in range(int(T[d, w, r])):
                                xg = get_xg(r, int(tile_xidx[tg]) // GT)
                                xs = (int(tile_xidx[tg]) % GT) * DP
                                rl = get_rel(tg // GT)
                                rs = (tg % GT) * DP
                                msg = mp.tile([P, DP], bf16, tag="msg")
                                nc.vector.tensor_tensor(
                                    out=msg[:], in0=xg[:, xs:xs + DP],
                                    in1=rl[:, rs:rs + DP],
                                    op=mybir.AluOpType.mult)
                                Sg = get_smat(tg // GT)
                                Ss = (tg % GT) * DP
                                nc.tensor.matmul(out=aggA[:], lhsT=msg[:, 0:128],
                                                 rhs=Sg[:, Ss:Ss + DP],
                                                 start=(k == 0),
                                                 stop=(k == ntile - 1))
                                nc.tensor.matmul(out=aggB[:], lhsT=msg[:, 128:D],
                                                 rhs=Sg[:, Ss:Ss + DP],
                                                 start=(k == 0),
                                                 stop=(k == ntile - 1))
                                k += 1
                                tg += 1
                        aggs.append((aggA, aggB))

                    # ---- window epilogue ----
                    # copy agg PSUM -> SBUF f32r
                    asb = []
                    for d in range(2):
                        cA = stg.tile([P, DP], f32r, tag="asbA")
                        cB = stg.tile([72, DP], f32r, tag="asbB")
                        nc.scalar.copy(out=cA[:], in_=aggs[d][0][:])
                        nc.scalar.copy(out=cB[:], in_=aggs[d][1][:])
                        asb.append((cA, cB))
                    # loop term: xT * loop_rel
                    lA = stg.tile([P, DP], f32r, tag="loopA")
                    lB = stg.tile([72, DP], f32r, tag="loopB")
                    for h in range(2):
                        xw = stg.tile([P, DP], bf16, tag="xw")
                        r0 = w * WIN + h * P
                        nc.sync.dma_start(out=xw[:], in_=xown[r0:r0 + P, :])
                        for (c0, csz), dstt in zip(KCH, (lA, lB)):
                            tp = ptp.tile([P, P], bf16, tag="ptp")
                            nc.tensor.transpose(out=tp[:csz, :],
                                                in_=xw[:, c0:c0 + csz],
                                                identity=ident_bf[:])
                            nc.scalar.mul(out=dstt[:csz, h * P:(h + 1) * P],
                                          in_=tp[:csz, :],
                                          mul=LR[l][0 if c0 == 0 else 1][:csz, :1])
                    # GEMMs: res = agg_in@Win + agg_out@Wout + loop@Wloop
                    terms = [(W[wi], asb[0]), (W[wo], asb[1]), (W[wl], (lA, lB))]
                    resP = []
                    for (m0, msz) in KCH:
                        op = pres.tile([P, DP], f32,
                                       tag=("presA" if m0 == 0 else "presB"))
                        first = True
                        for (Wt, rhs) in terms:
                            for ki, (k0, ksz) in enumerate(KCH):
                                nc.tensor.matmul(
                                    out=op[:msz, :],
                                    lhsT=Wt[ki][:, m0:m0 + msz],
                                    rhs=rhs[ki][:ksz, :],
                                    start=first,
                                    stop=(Wt is terms[2][0] and ki == 1))
                                first = False
                        resP.append(op)
                    # stash + stats
                    st = stats[l]
                    scr = stg.tile([P, DP], f32, tag="scr")
                    for (m0, msz), op, sname, qname, sd in zip(
                            KCH, resP, ("sA", "sB"), ("qA", "qB"),
                            (stashA, stashB)):
                        cpy = stg.tile([P, DP], f32, tag=f"stash{m0}")
                        nc.scalar.activation(
                            out=cpy[:msz, :], in_=op[:msz, :],
                            func=mybir.ActivationFunctionType.Copy,
                            accum_out=st[sname][:msz, w:w + 1])
                        nc.scalar.activation(
                            out=scr[:msz, :], in_=op[:msz, :],
                            func=mybir.ActivationFunctionType.Square,
                            accum_out=st[qname][:msz, w:w + 1])
                        nc.sync.dma_start(out=sd[w, 0:msz, :], in_=cpy[:msz, :])

                # ---------------- BN reduce + AllReduce ----------------
                st = stats[l]
                red = {}
                for nm, csz in [("sA", P), ("sB", 72), ("qA", P), ("qB", 72)]:
                    rt = cp.tile([P, 1], f32, tag=f"red{nm}{l}")
                    nc.vector.reduce_sum(out=rt[:csz, :1], in_=st[nm][:csz, :],
                                         axis=mybir.AxisListType.X)
                    red[nm] = rt
                nc.sync.dma_start(out=bn_in[l][0:128, 0:1], in_=red["sA"][:, :1])
                nc.sync.dma_start(out=bn_in[l][128:200, 0:1], in_=red["sB"][:72, :1])
                nc.sync.dma_start(out=bn_in[l][0:128, 1:2], in_=red["qA"][:, :1])
                nc.sync.dma_start(out=bn_in[l][128:200, 1:2], in_=red["qB"][:72, :1])
                nc.gpsimd.collective_compute(
                    "AllReduce", mybir.AluOpType.add, replica_groups=rg,
                    ins=[bn_in[l][:, :]], outs=[bn_out[l][:, :]])
                bn = {}
                for nm, (o0, csz, col) in {
                        "sA": (0, P, 0), "sB": (128, 72, 0),
                        "qA": (0, P, 1), "qB": (128, 72, 1)}.items():
                    rt = cp.tile([P, 1], f32, tag=f"bn{nm}{l}")
                    nc.sync.dma_start(out=rt[:csz, :1],
                                      in_=bn_out[l][o0:o0 + csz, col:col + 1])
                    bn[nm] = rt
                inv = {}
                nbias = {}
                for ch, csz in [("A", P), ("B", 72)]:
                    s_, q_ = bn["s" + ch], bn["q" + ch]
                    mu = cp.tile([P, 1], f32, tag=f"mu{ch}{l}")
                    nc.vector.tensor_scalar(out=mu[:csz], in0=s_[:csz],
                                            scalar1=1.0 / N, scalar2=None,
                                            op0=mybir.AluOpType.mult)
                    msq = cp.tile([P, 1], f32, tag=f"msq{ch}{l}")
                    nc.vector.tensor_scalar(out=msq[:csz], in0=q_[:csz],
                                            scalar1=1.0 / N, scalar2=None,
                                            op0=mybir.AluOpType.mult)
                    mu2 = cp.tile([P, 1], f32, tag=f"mu2{ch}{l}")
                    nc.vector.tensor_tensor(out=mu2[:csz], in0=mu[:csz],
                                            in1=mu[:csz],
                                            op=mybir.AluOpType.mult)
                    var = cp.tile([P, 1], f32, tag=f"var{ch}{l}")
                    nc.vector.tensor_tensor(out=var[:csz], in0=msq[:csz],
                                            in1=mu2[:csz],
                                            op=mybir.AluOpType.subtract)
                    std = cp.tile([P, 1], f32, tag=f"std{ch}{l}")
                    nc.scalar.activation(out=std[:csz], in_=var[:csz],
                                         func=mybir.ActivationFunctionType.Sqrt,
                                         bias=eps_t[:csz, :1])
                    iv = cp.tile([P, 1], f32, tag=f"inv{ch}{l}")
                    nc.vector.reciprocal(out=iv[:csz], in_=std[:csz])
                    t1 = cp.tile([P, 1], f32, tag=f"t1{ch}{l}")
                    nc.vector.tensor_tensor(out=t1[:csz], in0=mu[:csz],
                                            in1=iv[:csz],
                                            op=mybir.AluOpType.mult)
                    nb = cp.tile([P, 1], f32, tag=f"nb{ch}{l}")
                    nc.vector.tensor_scalar(out=nb[:csz], in0=t1[:csz],
                                            scalar1=-1.0, scalar2=None,
                                            op0=mybir.AluOpType.mult)
                    inv[ch] = iv
                    nbias[ch] = nb

                # ---------------- normalize + tanh + transpose out ----------
                odt = bf16 if l == 0 else f32
                ident = ident_bf if l == 0 else ident_f
                for w in range(NW):
                    ldA = stg.tile([P, DP], f32, tag="ldA")
                    ldB = stg.tile([72, DP], f32, tag="ldB")
                    nc.sync.dma_start(out=ldA[:], in_=stashA[w, :, :])
                    nc.sync.dma_start(out=ldB[:72], in_=stashB[w, :, :])
                    xpA = stg.tile([P, DP], odt, tag="xpA")
                    xpB = stg.tile([72, DP], odt, tag="xpB")
                    nc.scalar.activation(out=xpA[:], in_=ldA[:],
                                         func=mybir.ActivationFunctionType.Tanh,
                                         scale=inv["A"][:, :1],
                                         bias=nbias["A"][:, :1])
                    nc.scalar.activation(out=xpB[:72], in_=ldB[:72],
                                         func=mybir.ActivationFunctionType.Tanh,
                                         scale=inv["B"][:72, :1],
                                         bias=nbias["B"][:72, :1])
                    for h in range(2):
                        rowbase = w * WIN + h * P
                        nv = min(P, NSH - rowbase)
                        if nv <= 0:
                            continue
                        rows = rwp.tile([P, D], odt, tag="rows")
                        tpA = ptp.tile([P, P], odt, tag="ptp")
                        nc.tensor.transpose(out=tpA[:],
                                            in_=xpA[:, h * P:(h + 1) * P],
                                            identity=ident[:])
                        nc.scalar.copy(out=rows[:, 0:128], in_=tpA[:])
                        tpB = ptp.tile([P, P], odt, tag="ptp")
                        nc.tensor.transpose(out=tpB[:, :72],
                                            in_=xpB[:72, h * P:(h + 1) * P],
                                            identity=ident[:72, :72])
                        nc.scalar.copy(out=rows[:, 128:200], in_=tpB[:, :72])
                        if l == 0:
                            nc.sync.dma_start(out=ag_in[rowbase:rowbase + nv, 0:D],
                                              in_=rows[:nv, :])
                        else:
                            nc.sync.dma_start(out=x2_d[rowbase:rowbase + nv, :],
                                              in_=rows[:nv, :])

                if l == 0:
                    nc.gpsimd.collective_compute(
                        "AllGather", mybir.AluOpType.bypass, replica_groups=rg,
                        ins=[ag_in[0:NSH, :]], outs=[x1full[:, :]])

    nc.compile()
    return nc


_CACHE = {}


def kernel(**inputs):
    ei = np.asarray(inputs["edge_index"])
    et = np.asarray(inputs["edge_type"])
    meta, cores = _prep(ei, et)

    key = "prog"
    if key not in _CACHE:
        _CACHE[key] = _build(meta)
    nc = _CACHE[key]

    x0 = np.asarray(inputs["kg_node_emb"], np.float32)
    x0bf = np.zeros((N, DP), BF)
    x0bf[:, :D] = x0.astype(BF)
    rel1 = np.asarray(inputs["init_rel"], np.float32)
    rel2 = rel1 @ np.asarray(inputs["w_rel1"], np.float32)
    relbf = []
    for rl in (rel1, rel2):
        rb = np.zeros((R2, DP), BF)
        rb[:, :D] = rl.astype(BF)
        relbf.append(rb)

    shared = {
        "x0bf": x0bf,
        "win1": np.asarray(inputs["w_in1"], np.float32) / 3.0,
        "wout1": np.asarray(inputs["w_out1"], np.float32) / 3.0,
        "wloop1": np.asarray(inputs["w_loop1"], np.float32) / 3.0,
        "win2": np.asarray(inputs["w_in2"], np.float32) / 3.0,
        "wout2": np.asarray(inputs["w_out2"], np.float32) / 3.0,
        "wloop2": np.asarray(inputs["w_loop2"], np.float32) / 3.0,
        "lrel1": np.asarray(inputs["loop_rel1"], np.float32).reshape(D, 1),
        "lrel2": np.asarray(inputs["loop_rel2"], np.float32).reshape(D, 1),
    }
    in_maps = []
    for c in range(NCORE):
        m = dict(shared)
        x0own = np.zeros((NSH_PAD, DP), BF)
        x0own[:NSH] = x0bf[c * NSH:(c + 1) * NSH]
        m["x0own"] = x0own
        cd = cores[c]
        for r in range(NREG):
            m[f"xidx{r}"] = cd["xidx"][r]
        m["smat"] = cd["smat"]
        TOTG_PAD = cd["smat"].shape[1] // DP
        for l in range(2):
            g = relbf[l][cd["etrows"]]
            m[f"relg{l}"] = np.ascontiguousarray(
                g.reshape(TOTG_PAD, P, DP).transpose(1, 0, 2).reshape(P, -1))
        in_maps.append(m)

    trace = os.environ.get("COMPGCN_TRACE", "0") == "1"
    res = run_bass_kernel_spmd(nc, in_maps, core_ids=list(range(NCORE)),
                               trace=trace)
    if trace and res.exec_time_ns is not None:
        print(f"HW exec time: {res.exec_time_ns} ns")
        kernel.last_exec_ns = res.exec_time_ns

    x2 = np.concatenate([np.asarray(res.results[c]["x2"], np.float32)
                         for c in range(NCORE)], axis=0)
    org = np.asarray(inputs["org"]).astype(np.int64)
    return (x2[org], x2)


kernel.last_exec_ns = None
